# revision 26
# baseline (speedup 1.0000x reference)
"""BERT self-attention on 8 Trainium2 NeuronCores.

Problem: B=4, S=2048, H=768, nh=12, hd=64.
Sharding: core c -> (batch b = c//2, head-group g = c%2); each core does
1 batch x 6 heads: projections + attention + output slice [2048, 384].

v9 strategy (host does all data marshalling; fp8 DoubleRow on the
scores matmul only):
  - The host pre-transposes and pre-casts everything: x^T [768, 2048]
    bf16 (Q side), xp^T = x[perm]^T [768, KP] bf16 (K/V side,
    mask-compacted: unmasked k first), W^T [768, 384] bf16 per
    projection, and the exp mask bias [128, nt]. Every device load is
    a single batched contiguous DMA (x^T in two halves, separate tiles
    - dependency tracking is tile-granular, so a half-tile consumer
    must not share a tile with a later-arriving half), and the device
    does zero transposes: the output leaves as OUT^T[d, q] per head
    with the softmax-denominator row attached, and the host divides +
    transposes during unsharding.
  - Queue discipline: ACT carries only tiny const loads (a DMA in
    flight blocks the engine instructions behind it on that queue);
    SP carries the K-side loads, K repacks and output flushes; the
    SWDGE/Pool queue carries the Q/V-side loads and late repacks.
  - The attention mask depends only on k: masked k-columns contribute
    exactly 0 after exp(-10000) underflows, so the K/V extent shrinks
    from 16 to nt=9 k-blocks; nt=16 is the always-correct fallback on
    a conservative schedule (all projections staged serially).
  - Projections run in bf16 (fp8 projections would put ~2e-2 on the
    output); psum copy-outs quantize Q^T/K^T to flat fp8 (stage K on
    ACT, everything later on DVE), then stride-2 partition DMAs repack
    into the d-paired [32, 2, s] layout fp8 DoubleRow needs.
  - Scores: ST[k', q] = K^T.T @ Q^T as fp8e4 DoubleRow - half the PE
    time of bf16, ~1.2e-2 output error vs the 2e-2 budget. Mask folds
    into the exp() as a per-partition bias; no row-max subtraction.
  - V stays bf16 as [k', o] with a constant 1.0 column per head ->
    the P@V matmul's row 64 yields the softmax denominators.
  - Fast-path schedule: the stage projects only K-oc0 and Q-oc0-qh0,
    so the first exp fires ~16us in. Everything else - the other Q
    half and oc1/oc2 Q chunks, K oc1/oc2 chunks, the V projection -
    is poured one item per kb-slot into the exp-bound heads. P@V for
    head h-1 runs as per-qc sequential passes (one psum bank at a
    time, issued as half-passes) inside head h, which leaves psum
    banks free for the poured projection work; head h-1's pts all
    stay live through head h (pt pool 2*nt+1 deep). The last head
    finishes its predecessor's passes in its first slots, then
    accumulates its own PV kb-paced in 4 psum groups right behind its
    exp stream, so the post-exp tail is four copies and one flush.
  - PE warm-up transposes during the initial DMA window keep the
    tensor engine out of its low p-states when projections start.
"""

import numpy as np

import concourse.bacc as bacc
import concourse.bass as bass
import concourse.mybir as mybir
from concourse.bass_utils import run_bass_kernel_spmd
from concourse.masks import make_identity
from concourse.tile import TileContext

F32 = mybir.dt.float32
BF16 = mybir.dt.bfloat16
FP8 = mybir.dt.float8e4
DR = mybir.MatmulPerfMode.DoubleRow

S = 2048  # sequence length
H = 768  # hidden
O = 384  # per-core projection width (6 heads * 64)
HD = 64  # head dim
NHEADS = 6  # heads per core
NI = H // 128  # 6 contraction chunks
SB = S // 128  # 16 seq blocks
QC = S // 512  # 4 q chunks
NT_FAST = 9  # k-blocks kept in the compacted build (capacity 1152)
N_WARMUP = 40  # PE warm-up transposes to span the initial DMA window


def build_nc(nt):
    nc = bacc.Bacc(None, target_bir_lowering=False)

    KP = nt * 128
    xt_d = nc.dram_tensor("xt", [H, S], BF16, kind="ExternalInput")
    xpt_d = nc.dram_tensor("xpt", [H, KP], BF16, kind="ExternalInput")
    wqt_d = nc.dram_tensor("wqt", [H, O], BF16, kind="ExternalInput")
    wkt_d = nc.dram_tensor("wkt", [H, O], BF16, kind="ExternalInput")
    wvt_d = nc.dram_tensor("wvt", [H, O], BF16, kind="ExternalInput")
    bqc_d = nc.dram_tensor("bqc", [128, 3], F32, kind="ExternalInput")
    bkc_d = nc.dram_tensor("bkc", [128, 3], F32, kind="ExternalInput")
    bvr_d = nc.dram_tensor("bvr", [1, O], BF16, kind="ExternalInput")
    mb_d = nc.dram_tensor("mb", [128, nt], F32, kind="ExternalInput")
    # per-head OUT^T with the softmax-denominator row; host normalizes
    out = nc.dram_tensor("out", [NHEADS, HD + 1, S], F32, kind="ExternalOutput")

    with nc.allow_low_precision("bf16/fp8 activations by design"), TileContext(nc) as tc:
        _body(nc, tc, nt, xt_d, xpt_d, wqt_d, wkt_d, wvt_d,
              bqc_d, bkc_d, bvr_d, mb_d, out)

    nc.finalize()
    return nc


def _body(nc, tc, nt, xt_d, xpt_d, wqt_d, wkt_d, wvt_d,
          bqc_d, bkc_d, bvr_d, mb_d, out):
    from contextlib import ExitStack

    Exp = mybir.ActivationFunctionType.Exp
    Ident = mybir.ActivationFunctionType.Identity
    KP = nt * 128
    fast = nt == NT_FAST
    # 512-wide projection chunks for the poured K work
    k512 = []
    off = 0
    while off < KP:
        w = min(512, KP - off)
        k512.append((off, w))
        off += w

    def by_chunk(dram):  # [768, F] -> [128, 6, F] batched-load view
        return dram.rearrange("(c p) f -> p c f", p=128)

    with ExitStack() as ctx:
        consts = ctx.enter_context(tc.tile_pool(name="consts", bufs=1))
        identity = consts.tile([128, 128], F32, tag="identity")
        make_identity(nc, identity)

        ones_row = consts.tile([1, 128], BF16, tag="ones_row")
        nc.vector.memset(ones_row, 1.0)

        bqcol = consts.tile([128, 3], F32, tag="bqcol")
        bkcol = consts.tile([128, 3], F32, tag="bkcol")
        bvrow = consts.tile([1, O], BF16, tag="bvrow")
        mask_bias = consts.tile([128, nt], F32, tag="mask_bias")

        # persistent activation tiles
        qkvp = ctx.enter_context(tc.tile_pool(name="qkv", bufs=1))
        qtp = [qkvp.tile([64, 2, S], FP8, tag=f"qtp{i}", name=f"qtp{i}") for i in range(3)]
        ktp = [qkvp.tile([64, 2, KP], FP8, tag=f"ktp{i}", name=f"ktp{i}") for i in range(3)]
        vt = [
            qkvp.tile([128, NHEADS, HD + 1], BF16, tag=f"v{i}", name=f"v{i}")
            for i in range(nt)
        ]
        outt_pool = ctx.enter_context(tc.tile_pool(name="outt", bufs=2))
        small = ctx.enter_context(tc.tile_pool(name="small", bufs=4))

        # tiles that live past the stage phase (fast path pours most
        # projection work into the attention heads)
        stage2 = ctx.enter_context(tc.tile_pool(name="stage2", bufs=1))
        # xp^T split so the first K chunk (k' 0:512) doesn't wait the
        # full load (dependency tracking is tile-granular)
        xptC0 = stage2.tile([128, NI, 512], BF16, tag="xptC0")
        xptC1 = stage2.tile([128, NI, KP - 512], BF16, tag="xptC1")
        wtvC = stage2.tile([128, NI, O], BF16, tag="wtvC")
        wtqC = stage2.tile([128, NI, O], BF16, tag="wtqC")
        # x^T halves in separate tiles: dependency tracking is
        # tile-granular and the qh0 projection must not wait on the
        # second half's DMA
        xtC0 = stage2.tile([128, NI, 1024], BF16, tag="xtC0")
        xtC1 = stage2.tile([128, NI, 1024], BF16, tag="xtC1")
        qt8 = [stage2.tile([128, S], FP8, tag=f"qt8{i}", name=f"qt8{i}") for i in range(3)]
        if fast:
            wtkC = stage2.tile([128, NI, O], BF16, tag="wtkC")
            kt8 = [stage2.tile([128, KP], FP8, tag=f"kt8{i}", name=f"kt8{i}") for i in range(3)]

        def xt_slice(i, lo, w):  # [128, w] slice of x^T chunk i at col lo
            t = xtC0 if lo < 1024 else xtC1
            return t[:, i, lo % 1024 : lo % 1024 + w]

        def xpt_slice(i, lo, w):  # [128, w] slice of xp^T chunk i at col lo
            if lo < 512:
                return xptC0[:, i, lo : lo + w]
            return xptC1[:, i, lo - 512 : lo - 512 + w]

        def qproj_chunk(pool, oc, qc, rep=None):
            # one [128, 512] Q^T projection chunk + fp8 copy-out on DVE
            ps = pool.tile([128, 512], F32, tag="pv", name="qps")
            for i in range(NI):
                nc.tensor.matmul(
                    ps,
                    wtqC[:, i, oc * 128 : (oc + 1) * 128],
                    xt_slice(i, qc * 512, 512),
                    start=(i == 0),
                    stop=(i == NI - 1),
                )
            nc.vector.tensor_scalar_add(
                qt8[oc][:, qc * 512 : (qc + 1) * 512], ps, bqcol[:, oc : oc + 1]
            )
            if rep:
                rep()

        def qrepack(oc, lo=0, hi=S, eng=None):
            eng = eng or nc.sync
            for s_ in range(2):
                eng.dma_start(qtp[oc][:, s_, lo:hi], qt8[oc][s_:128:2, lo:hi])

        # ---- stage phase ----
        with (
            tc.tile_pool(name="stage", bufs=1) as stage,
            tc.tile_pool(name="psA", bufs=4, space="PSUM") as psA,
        ):
            if not fast:
                wtkC = stage.tile([128, NI, O], BF16, tag="wtkCs")
                kt8 = [stage.tile([128, KP], FP8, tag=f"kt8{i}", name=f"kt8{i}") for i in range(3)]

            # ACT queue: only the small consts.
            nc.scalar.dma_start(mask_bias, mb_d[:, :])
            nc.scalar.dma_start(bkcol, bkc_d[:, :])
            nc.scalar.dma_start(bqcol, bqc_d[:, :])
            # SP queue: Q-side loads (they gate the first scores).
            xtv = by_chunk(xt_d)
            xpv = by_chunk(xpt_d)
            nc.sync.dma_start(wtqC, by_chunk(wqt_d))
            nc.sync.dma_start(xtC0, xtv[:, :, 0:1024])
            # SWDGE/Pool queue: K-side loads. The second x^T half and
            # the V-side loads are issued after the stage projections so
            # the first K repack isn't stuck behind them in the FIFO DMA
            # device (fast path only).
            nc.gpsimd.dma_start(wtkC, by_chunk(wkt_d))
            nc.gpsimd.dma_start(xptC0, xpv[:, :, 0:512])
            nc.gpsimd.dma_start(xptC1, xpv[:, :, 512:KP])
            if not fast:
                nc.gpsimd.dma_start(xtC1, xtv[:, :, 1024:S])
                nc.gpsimd.dma_start(wtvC, by_chunk(wvt_d))
                nc.gpsimd.dma_start(bvrow, bvr_d[:, :])

            # PE warm-up + Exp table preload
            warm = psA.tile([128, 1024], F32, tag="ps")
            for w in range(N_WARMUP):
                nc.tensor.transpose(
                    warm[:, (w % 8) * 128 : (w % 8 + 1) * 128], identity, identity
                )
            exp_warm = small.tile([1, 1], F32, tag="expw", name="expw")
            nc.scalar.activation(exp_warm, identity[0:1, 0:1], Exp)

            def kproj_stage(oc):
                # staged K oc, per-512 chunks with incremental repacks so
                # the first scores only wait on k' 0:512: chunk0's copy on
                # ACT (free then), later chunk copies on DVE so the ACT
                # queue stays clear for the first exps
                for ci, (coff, cw) in enumerate(k512):
                    ps = psA.tile([128, 1024], F32, tag="ps")
                    for i in range(NI):
                        nc.tensor.matmul(
                            ps[:, 0:cw],
                            wtkC[:, i, oc * 128 : (oc + 1) * 128],
                            xpt_slice(i, coff, cw),
                            start=(i == 0),
                            stop=(i == NI - 1),
                        )
                    if ci == 0:
                        nc.scalar.activation(
                            kt8[oc][:, coff : coff + cw],
                            ps[:, 0:cw],
                            Ident,
                            bias=bkcol[:, oc : oc + 1],
                        )
                    else:
                        nc.vector.tensor_scalar_add(
                            kt8[oc][:, coff : coff + cw],
                            ps[:, 0:cw],
                            bkcol[:, oc : oc + 1],
                        )
                    eng = nc.gpsimd if (fast and ci == 0) else nc.sync
                    for s_ in range(2):
                        eng.dma_start(
                            ktp[oc][:, s_, coff : coff + cw],
                            kt8[oc][s_:128:2, coff : coff + cw],
                        )

            def qproj_stage(oc, qh):
                # staged Q half: [128, 1024] psum, DVE copy-out
                ps = psA.tile([128, 1024], F32, tag="ps")
                for qq in range(2):
                    qcc = qh * 2 + qq
                    for i in range(NI):
                        nc.tensor.matmul(
                            ps[:, qq * 512 : (qq + 1) * 512],
                            wtqC[:, i, oc * 128 : (oc + 1) * 128],
                            xt_slice(i, qcc * 512, 512),
                            start=(i == 0),
                            stop=(i == NI - 1),
                        )
                nc.vector.tensor_scalar_add(
                    qt8[oc][:, qh * 1024 : (qh + 1) * 1024], ps, bqcol[:, oc : oc + 1]
                )

            if fast:
                # minimal prefix: only what head 0's first scores need.
                # K first - its operands land first, and a PE idle gap
                # before a projection resets the p-state ramp.
                qproj_stage(0, 0)
                qrepack(0, 0, 1024, eng=nc.scalar)
                kproj_stage(0)
                # deferred Q/V-side loads, behind the first K repack
                nc.gpsimd.dma_start(xtC1, xtv[:, :, 1024:S])
                nc.gpsimd.dma_start(wtvC, by_chunk(wvt_d))
                nc.gpsimd.dma_start(bvrow, bvr_d[:, :])
            else:
                # conservative fallback: stage everything
                for oc in range(3):
                    kproj_stage(oc)
                for oc in range(3):
                    for qh in range(2):
                        qproj_stage(oc, qh)
                    qrepack(oc)

        # ---- attention ----
        with (
            tc.tile_pool(name="pt", bufs=(2 * nt + 1) if fast else nt + 4) as ptp,
            tc.tile_pool(name="st", bufs=2, space="PSUM") as stp,
            tc.tile_pool(name="pv", bufs=4, space="PSUM") as pvp,
        ):
            def st_exp(kt_h, qt_h, pt, kb, qh):
                st = stp.tile([128, 1024], F32, tag="st", name="st")
                for qq in range(2):
                    qcc = qh * 2 + qq
                    nc.tensor.matmul(
                        st[:, qq * 512 : (qq + 1) * 512],
                        kt_h[:, :, kb * 128 : (kb + 1) * 128],
                        qt_h[:, :, qcc * 512 : (qcc + 1) * 512],
                        start=True,
                        stop=True,
                        perf_mode=DR,
                    )
                nc.scalar.activation(
                    pt[:, qh * 1024 : (qh + 1) * 1024],
                    st,
                    Exp,
                    bias=mask_bias[:, kb : kb + 1],
                    scale=0.125,
                )

            def vproj(kb):
                psv = pvp.tile([128, 512], F32, tag="pv", name="psv")
                for i in range(NI):
                    nc.tensor.matmul(
                        psv[:, 0:O],
                        xpt_slice(i, kb * 128, 128),
                        wtvC[:, i, :],
                        start=(i == 0),
                        stop=False,
                    )
                nc.tensor.matmul(psv[:, 0:O], ones_row, bvrow, start=False, stop=True)
                nc.vector.tensor_copy(
                    vt[kb][:, :, 0:HD],
                    psv[:, 0:O].rearrange("p (h d) -> p h d", d=HD),
                )
                nc.vector.memset(vt[kb][:, :, HD : HD + 1], 1.0)

            def koc_chunk(oc, ci):
                # poured K projection chunk (512 wide), DVE copy-out
                coff, cw = k512[ci]
                ps = pvp.tile([128, 512], F32, tag="pv", name="kps")
                for i in range(NI):
                    nc.tensor.matmul(
                        ps[:, 0:cw],
                        wtkC[:, i, oc * 128 : (oc + 1) * 128],
                        xpt_slice(i, coff, cw),
                        start=(i == 0),
                        stop=(i == NI - 1),
                    )
                nc.vector.tensor_scalar_add(
                    kt8[oc][:, coff : coff + cw], ps[:, 0:cw], bkcol[:, oc : oc + 1]
                )
                if ci == len(k512) - 1:
                    for s_ in range(2):
                        nc.gpsimd.dma_start(ktp[oc][:, s_, :], kt8[oc][s_:128:2, :])

            def flush_head(hp, outt):
                nc.sync.dma_start(out[hp], outt)

            # fast-path poured work items, in dependency-safe order
            if fast:
                items = []
                items.append(lambda: qproj_chunk(pvp, 0, 2))
                items.append(lambda: qproj_chunk(pvp, 0, 3, rep=lambda: qrepack(0, 1024, S)))
                for qc in range(QC):
                    rep = (lambda: qrepack(1)) if qc == QC - 1 else None
                    items.append(lambda qc=qc, rep=rep: qproj_chunk(pvp, 1, qc, rep=rep))
                for kb in range(nt):
                    items.append(lambda kb=kb: vproj(kb))
                for ci in range(len(k512)):
                    items.append(lambda ci=ci: koc_chunk(1, ci))
                itemsB = []
                for qc in range(QC):
                    rep = (lambda: qrepack(2)) if qc == QC - 1 else None
                    itemsB.append(lambda qc=qc, rep=rep: qproj_chunk(pvp, 2, qc, rep=rep))
                for ci in range(len(k512)):
                    itemsB.append(lambda ci=ci: koc_chunk(2, ci))
                items = list(reversed(items))  # pop from the end
                itemsB = list(reversed(itemsB))

            def pop_item(q):
                if q:
                    q.pop()()

            prev = None  # (head, pts, outt) being passed/drained

            for h in range(NHEADS):
                oc, hh = h // 2, h % 2
                qt_h = qtp[oc][hh * 32 : hh * 32 + 32, :, :]
                kt_h = ktp[oc][hh * 32 : hh * 32 + 32, :, :]
                last_head = h == NHEADS - 1

                pts = []
                if fast and h == 0:
                    # two q-half passes; pour items from slot 2 on
                    for kb in range(nt):
                        pt = ptp.tile([128, S], BF16, tag="pt", name="pt")
                        pts.append(pt)
                        st_exp(kt_h, qt_h, pt, kb, 0)
                        if kb >= 2:
                            pop_item(items)
                    for kb in range(nt):
                        st_exp(kt_h, qt_h, pts[kb], kb, 1)
                        pop_item(items)
                elif fast and not last_head:
                    # steady head: ST/exp stream + one poured item per
                    # slot + half-passes of head h-1's PV on slots 1..8
                    hp, pts_p, outt_p = prev
                    Tq = [None] * QC
                    for kb in range(nt):
                        pt = ptp.tile([128, S], BF16, tag="pt", name="pt")
                        pts.append(pt)
                        st_exp(kt_h, qt_h, pt, kb, 0)
                        st_exp(kt_h, qt_h, pt, kb, 1)
                        # poured item (h1 keeps pouring `items`, h2
                        # pours the B list, h3 pours what's left)
                        if h == 1:
                            pop_item(items)
                        elif h >= 2:
                            pop_item(itemsB)
                        # half-passes: slots 1..8 -> (qc, half) pairs
                        if 1 <= kb <= 8:
                            qc, half = (kb - 1) // 2, (kb - 1) % 2
                            if half == 0:
                                Tq[qc] = pvp.tile([128, 512], F32, tag="pv", name="pvT")
                            lo = 0 if half == 0 else (nt + 1) // 2
                            hi = (nt + 1) // 2 if half == 0 else nt
                            for kk in range(lo, hi):
                                nc.tensor.matmul(
                                    Tq[qc][0 : HD + 1, :],
                                    vt[kk][:, hp, :],
                                    pts_p[kk][:, qc * 512 : (qc + 1) * 512],
                                    start=(kk == 0),
                                    stop=(kk == nt - 1),
                                )
                            if half == 1:
                                nc.vector.tensor_copy(
                                    outt_p[:, qc * 512 : (qc + 1) * 512],
                                    Tq[qc][0 : HD + 1, :],
                                )
                                if qc == QC - 1:
                                    flush_head(hp, outt_p)
                elif fast and last_head:
                    # last head: finish h-1's passes in slots 0..3, then
                    # kb-paced 4-group PV for ourselves from slot 5
                    hp, pts_p, outt_p = prev
                    pvg = None
                    done = 0
                    for kb in range(nt):
                        pt = ptp.tile([128, S], BF16, tag="pt", name="pt")
                        pts.append(pt)
                        st_exp(kt_h, qt_h, pt, kb, 0)
                        st_exp(kt_h, qt_h, pt, kb, 1)
                        if kb < QC:
                            qc = kb
                            T = pvp.tile([128, 512], F32, tag="pv", name="pvT")
                            for kk in range(nt):
                                nc.tensor.matmul(
                                    T[0 : HD + 1, :],
                                    vt[kk][:, hp, :],
                                    pts_p[kk][:, qc * 512 : (qc + 1) * 512],
                                    start=(kk == 0),
                                    stop=(kk == nt - 1),
                                )
                            nc.vector.tensor_copy(
                                outt_p[:, qc * 512 : (qc + 1) * 512], T[0 : HD + 1, :]
                            )
                            if qc == QC - 1:
                                flush_head(hp, outt_p)
                        elif kb >= 5 or nt > NT_FAST:
                            if pvg is None:
                                pvg = [
                                    pvp.tile([128, 512], F32, tag="pv", name="pvg")
                                    for _ in range(QC)
                                ]
                            target = min(nt, 3 * (kb - 4), kb + 1)
                            while done < target:
                                for qc in range(QC):
                                    nc.tensor.matmul(
                                        pvg[qc][0 : HD + 1, :],
                                        vt[done][:, h, :],
                                        pts[done][:, qc * 512 : (qc + 1) * 512],
                                        start=(done == 0),
                                        stop=(done == nt - 1),
                                    )
                                done += 1
                else:
                    # conservative fallback: kb-major PV pipeline, V in
                    # head 0's slack
                    if prev is not None:
                        hp, pts_p, outt_p = prev
                        pvg = [
                            pvp.tile([128, 512], F32, tag="pv", name="pvg")
                            for _ in range(QC)
                        ]
                    for kb in range(nt):
                        pt = ptp.tile([128, S], BF16, tag="pt", name="pt")
                        pts.append(pt)
                        st_exp(kt_h, qt_h, pt, kb, 0)
                        st_exp(kt_h, qt_h, pt, kb, 1)
                        if h == 0:
                            vproj(kb)
                        if prev is not None:
                            for qc in range(QC):
                                nc.tensor.matmul(
                                    pvg[qc][0 : HD + 1, :],
                                    vt[kb][:, hp, :],
                                    pts_p[kb][:, qc * 512 : (qc + 1) * 512],
                                    start=(kb == 0),
                                    stop=(kb == nt - 1),
                                )
                    if prev is not None:
                        for qc in range(QC):
                            nc.vector.tensor_copy(
                                outt_p[:, qc * 512 : (qc + 1) * 512],
                                pvg[qc][0 : HD + 1, :],
                            )
                        flush_head(hp, outt_p)

                outt = outt_pool.tile([HD + 1, S], F32, tag="outt", name="outt")
                prev = (h, pts, outt)

            # tail
            hp, pts_p, outt_p = prev
            if fast:
                while done < nt:
                    for qc in range(QC):
                        nc.tensor.matmul(
                            pvg[qc][0 : HD + 1, :],
                            vt[done][:, hp, :],
                            pts_p[done][:, qc * 512 : (qc + 1) * 512],
                            start=(done == 0),
                            stop=(done == nt - 1),
                        )
                    done += 1
            else:
                pvg = [
                    pvp.tile([128, 512], F32, tag="pv", name="pvg") for _ in range(QC)
                ]
                for qc in range(QC):
                    for kb in range(nt):
                        nc.tensor.matmul(
                            pvg[qc][0 : HD + 1, :],
                            vt[kb][:, hp, :],
                            pts_p[kb][:, qc * 512 : (qc + 1) * 512],
                            start=(kb == 0),
                            stop=(kb == nt - 1),
                        )
            # exposed last drain: copies split DVE/ACT, flush per half
            for qc in range(QC):
                sl = slice(qc * 512, (qc + 1) * 512)
                if qc % 2 == 1:
                    nc.scalar.activation(outt_p[:, sl], pvg[qc][0 : HD + 1, :], Ident)
                else:
                    nc.vector.tensor_copy(outt_p[:, sl], pvg[qc][0 : HD + 1, :])
                if qc == 1:
                    nc.sync.dma_start(out[hp][:, 0:1024], outt_p[:, 0:1024])
            nc.sync.dma_start(out[hp][:, 1024:S], outt_p[:, 1024:S])


_NC_CACHE = {}


def _get_nc(nt):
    if nt not in _NC_CACHE:
        _NC_CACHE[nt] = build_nc(nt)
    return _NC_CACHE[nt]


def _make_in_maps(inputs, nt):
    import ml_dtypes

    bf16 = ml_dtypes.bfloat16
    KP = nt * 128
    hs = np.asarray(inputs["hidden_states"], dtype=np.float32)
    am = np.asarray(inputs["attention_mask"], dtype=np.float32)
    Wq = np.asarray(inputs["Wq"], dtype=np.float32)
    Wk = np.asarray(inputs["Wk"], dtype=np.float32)
    Wv = np.asarray(inputs["Wv"], dtype=np.float32)
    bq = np.asarray(inputs["bq"], dtype=np.float32)
    bk = np.asarray(inputs["bk"], dtype=np.float32)
    bv = np.asarray(inputs["bv"], dtype=np.float32)

    xt_b, xpt_b, mb_b = [], [], []
    for b in range(4):
        m = am[b, 0, 0, :]
        keep = np.nonzero(m >= 0)[0]
        drop = np.nonzero(m < 0)[0]
        perm = np.concatenate([keep, drop])[:KP]
        xt_b.append(np.ascontiguousarray(hs[b].T.astype(bf16)))
        xpt_b.append(np.ascontiguousarray(hs[b][perm].T.astype(bf16)))
        mbias = np.where(m[perm] < 0, np.float32(-10000.0), np.float32(0.0))
        mb_b.append(np.ascontiguousarray(mbias.reshape(nt, 128).T))

    in_maps = []
    for c in range(8):
        b, g = c // 2, c % 2
        sl = slice(g * O, (g + 1) * O)
        in_maps.append(
            {
                "xt": xt_b[b],
                "xpt": xpt_b[b],
                "mb": mb_b[b],
                "wqt": np.ascontiguousarray(Wq[sl].T.astype(bf16)),
                "wkt": np.ascontiguousarray(Wk[sl].T.astype(bf16)),
                "wvt": np.ascontiguousarray(Wv[sl].T.astype(bf16)),
                "bqc": np.ascontiguousarray(bq[sl].reshape(3, 128).T),
                "bkc": np.ascontiguousarray(bk[sl].reshape(3, 128).T),
                "bvr": np.ascontiguousarray(bv[sl].astype(bf16)[None, :]),
            }
        )
    return in_maps


def _assemble(results):
    # device returns per-head OUT^T [6, 65, 2048]: rows 0..63 are the
    # unnormalized output, row 64 the softmax denominator; divide and
    # transpose while unsharding
    outp = np.empty((4, S, H), dtype=np.float32)
    for c in range(8):
        b, g = c // 2, c % 2
        raw = results[c]["out"]
        num = raw[:, 0:HD, :]  # [6, 64, S]
        den = raw[:, HD : HD + 1, :]  # [6, 1, S]
        o = np.transpose(num / den, (2, 0, 1)).reshape(S, O)
        outp[b, :, g * O : (g + 1) * O] = o
    return outp


def _pick_nt(inputs):
    am = np.asarray(inputs["attention_mask"], dtype=np.float32)
    max_keep = int((am[:, 0, 0, :] >= 0).sum(axis=1).max())
    return NT_FAST if max_keep <= NT_FAST * 128 else SB


def kernel(**inputs):
    nt = _pick_nt(inputs)
    nc = _get_nc(nt)
    in_maps = _make_in_maps(inputs, nt)
    res = run_bass_kernel_spmd(nc, in_maps, core_ids=list(range(8)))
    return _assemble(res.results)
